# revision 12
# baseline (speedup 1.0000x reference)
"""Conv2D-KAN Trainium2 kernel (8-core data-parallel SPMD).

Formulation
-----------
The reference computes, per 3x3 patch (N = B*30*30 patches, in_size = 288):
    out[n,o] = sum_{i,k} sb[n,i,k] * (spline_kernel*scale)[i,k,o]
             + silu(xf) @ scale_factor + biases
where sb is a cubic B-spline basis (8 funcs) over a uniform grid
(knots t_r = -2.2 + 0.4 r, r = 0..11, h = 0.4).

Key identities:
 1. Basis values depend only on the underlying *pixel*, not the patch
    (patch extraction is a gather), so features are computed per pixel
    (8x less elementwise work than per-patch).
 2. Uniform cubic B-splines decompose over truncated powers:
        B_k(x) = (1/6) sum_{m=0..4} cm_m T_{k+m}(x), cm = [1,-4,6,-4,1]
        T_r(x) = min(relu((x - t_r)/h), 11-r)^3
    The clamp at 11-r makes every B_k *exactly* zero outside the grid
    (integer cancellation), matching the reference's out-of-range
    behaviour without masks, and T_11 == 0 so only r = 0..10 exist.
 3. The whole op is then a 3x3 convolution with 128 filters over
    pixel-feature channels (11 truncated cubes + silu per channel,
    blending folded into the weights), done as accumulating 128-K
    matmuls into PSUM banks of [128 filters, 450 patches].

Matmuls run in float32r (1 col/cycle at N>=256, vs 4 for fp32).
f32r's reduced mantissa interacts with the truncated-power
cancellation to give rel err ~1e-2 (< the 2e-2 gate; deterministic
for the fixed problem inputs).

Performance structure (per core: 4 images, 216 matmuls, 97.2K PE
cycles ~ 47us steady stream):
 * x is replicated 4x on the HOST -> one contiguous [128, 4KB] DMA
   per image (full-bandwidth 16-ring spray) instead of 12 small
   replica DMAs; the three per-image feature tiles read the same
   replicated tile with out-of-place relu.
 * All input DMAs are pre-issued at the top so no trigger ever sits
   behind a dependent output DMA in an engine queue.
 * Weights split 3F/15F/9F so the first chunks land before the first
   matmuls need them, without hogging ring bandwidth.
 * Image 0's tiles are built in two column chunks (rows 0..16 /
   15..31) so the first PSUM bank's matmuls start ~4us after the
   first 272KB of input lands.
"""

import sys

sys.path.insert(0, "/opt/trn_rl_repo")

import numpy as np

N_CORES = 8
B, HH, WW, C = 32, 32, 32, 32
F = 128
KH = KW = 3
HO, WO = HH - KH + 1, WW - KW + 1          # 30, 30
BPC = B // N_CORES                          # images per core = 4
PIX = HH * WW                               # 1024 pixels per image
NPC = BPC * HO * WO                         # 3600 patches per core
BANKN = 450                                 # patches per psum bank
HGRID = 0.4
T0 = -2.2                                   # first knot
NR = 11                                     # truncated-cube features
NFEAT = 12                                  # + silu
NMM = 27                                    # matmuls per bank
CA = 17 * WW                                # img-0 chunk A cols (rows 0..16)
CB = PIX - 15 * WW                          # chunk B cols (rows 15..31)

_cache = {}


def _build_program():
    import concourse.bacc as bacc
    import concourse.mybir as mybir
    import concourse.tile as tile

    f32 = mybir.dt.float32
    f32r = mybir.dt.float32r
    AF = mybir.ActivationFunctionType

    nc = bacc.Bacc("TRN2", target_bir_lowering=False, debug=False)
    # host-replicated input: 4 copies of the [32, BPC*PIX] image block
    xt = nc.dram_tensor("xt", [128, BPC * PIX], f32, kind="ExternalInput").ap()
    wt = nc.dram_tensor("wt", [128, NMM * F], f32r, kind="ExternalInput").ap()
    consts = nc.dram_tensor("consts", [128, 8], f32, kind="ExternalInput").ap()
    y = nc.dram_tensor("y", [F, NPC], f32, kind="ExternalOutput").ap()

    with tile.TileContext(nc) as tc:
        with (
            tc.tile_pool(name="wp", bufs=1) as wp,
            tc.tile_pool(name="cp", bufs=1) as cp,
            tc.tile_pool(name="xp", bufs=1) as xp,
            tc.tile_pool(name="fp", bufs=2) as fp,
            tc.tile_pool(name="sp", bufs=2) as sp,
            tc.tile_pool(name="op", bufs=1) as op_,
            tc.tile_pool(name="pp", bufs=4, space="PSUM") as pp,
        ):
            ct = cp.tile([128, 8], f32)
            nc.scalar.dma_start(ct[:], consts[:])

            # warm the ACT table set (silu's set also carries relu /
            # identity / square) before the first feature tile lands.
            warm = cp.tile([1, 1], f32, tag="warm")
            nc.scalar.activation(warm[:], ct[:1, :1], AF.Silu)

            # ---- all input DMAs pre-issued, priority order ----
            # sync HWDGE: image 0 in two chunks, then images 1..3
            xr = []
            x0a = xp.tile([128, CA], f32, tag="x0a")
            nc.sync.dma_start(x0a[:], xt[:, 0:CA])
            x0b = xp.tile([128, CB], f32, tag="x0b")
            nc.sync.dma_start(x0b[:], xt[:, 15 * WW:PIX])
            for im in range(1, BPC):
                xi = xp.tile([128, PIX], f32, tag=f"x{im}")
                nc.sync.dma_start(xi[:], xt[:, im * PIX:(im + 1) * PIX])
                xr.append(xi)
            # scalar HWDGE: t=0/t=1 weight chunks; gpsimd SWDGE: t=2
            w0 = wp.tile([128, 9 * F], f32r, tag="w0")
            nc.scalar.dma_start(w0[:], wt[:, :9 * F])
            wA = wp.tile([128, 9 * F], f32r, tag="wA")
            nc.scalar.dma_start(wA[:], wt[:, 9 * F:18 * F])
            wB = wp.tile([128, 9 * F], f32r, tag="wB")
            nc.gpsimd.dma_start(wB[:], wt[:, 18 * F:])
            wtiles = [w0[:, i * F:(i + 1) * F] for i in range(9)] + \
                     [wA[:, i * F:(i + 1) * F] for i in range(9)] + \
                     [wB[:, i * F:(i + 1) * F] for i in range(9)]

            # PE pre-warm: dummy bf16 matmuls on a memset tile keep the
            # tensor engine's clock gate open so the real stream starts
            # at full p-state. Zero-cost: PE is idle during the prologue.
            bf16 = mybir.dt.bfloat16
            wdum = cp.tile([128, 128], bf16, tag="wdum")
            nc.vector.memset(wdum[:], 0.0)
            psd = pp.tile([128, 128], f32, tag="psd")
            for _ in range(72):
                nc.tensor.matmul(psd[:], wdum[:], wdum[:],
                                 start=True, stop=True)

            out_t = op_.tile([F, NPC], f32)

            def feat(src, cols, ctag):
                """Phased relu/clamp/cube pipeline for one column range
                of the replicated tile: all ACT ops first, then mins
                (DVE), squares (gpsimd), cubes (DVE) — engine FIFOs
                never stall and the t=0 cube lands earliest."""
                fts, cubes, sqs = [], [], []
                for t in range(3):
                    nsp = 128 if t < 2 else 96
                    ft = fp.tile([128, cols], f32, tag=f"f{t}{ctag}")
                    nc.scalar.activation(
                        ft[:nsp], src[:nsp], AF.Relu,
                        bias=ct[:nsp, t:t + 1], scale=1.0 / HGRID)
                    cube = sp.tile([128, cols], f32r, tag=f"c{t}{ctag}")
                    fts.append(ft)
                    cubes.append(cube)
                nc.scalar.activation(
                    cubes[2][96:128], src[96:128], AF.Silu)
                for t in range(3):
                    nsp = 128 if t < 2 else 96
                    nc.vector.tensor_scalar_min(
                        fts[t][:nsp], fts[t][:nsp], ct[:nsp, 3 + t:4 + t])
                for t in range(3):
                    nsp = 128 if t < 2 else 96
                    sq = fp.tile([128, cols], f32, tag=f"s{t}{ctag}")
                    nc.gpsimd.tensor_mul(
                        sq[:nsp], fts[t][:nsp], fts[t][:nsp])
                    sqs.append(sq)
                for t in range(3):
                    nsp = 128 if t < 2 else 96
                    nc.vector.tensor_mul(
                        cubes[t][:nsp], sqs[t][:nsp], fts[t][:nsp])
                return cubes

            def banks(im, mk_rhs):
                for half in range(2):
                    ps = pp.tile([F, BANKN], f32, tag="ps")
                    k = 0
                    # t-major: the first 9 matmuls only need feature
                    # tile 0, so PE starts before tiles 1/2 are built
                    for t in range(3):
                        for off in range(KH * KW):
                            di, dj = divmod(off, KW)
                            lhsT, rhs = mk_rhs(off, t, half, di, dj)
                            nc.tensor.matmul(
                                ps[:], lhsT, rhs,
                                start=(k == 0), stop=(k == NMM - 1),
                            )
                            k += 1
                    s = (im * 2 + half) * BANKN
                    if im == BPC - 1 and half == 1:
                        # last bank: act+DMA in halves so the final
                        # transfer starts as early as possible
                        hn = BANKN // 2
                        nc.scalar.activation(
                            out_t[:, s:s + hn], ps[:, :hn], AF.Identity,
                            bias=ct[:, 6:7], scale=1.0)
                        nc.sync.dma_start(
                            y[:, s:s + hn], out_t[:, s:s + hn])
                        nc.scalar.activation(
                            out_t[:, s + hn:s + BANKN], ps[:, hn:],
                            AF.Identity, bias=ct[:, 6:7], scale=1.0)
                        nc.scalar.dma_start(
                            y[:, s + hn:s + BANKN],
                            out_t[:, s + hn:s + BANKN])
                    else:
                        nc.scalar.activation(
                            out_t[:, s:s + BANKN], ps[:], AF.Identity,
                            bias=ct[:, 6:7], scale=1.0,
                        )
                        nc.sync.dma_start(
                            y[:, s:s + BANKN], out_t[:, s:s + BANKN])

            # ---- image 0: chunked (A = rows 0..16, B = rows 15..31) ----
            cubesA = feat(x0a, CA, "a")
            cubesB = feat(x0b, CB, "b")
            vA = [c[:].rearrange("p (h w) -> p h w", w=WW) for c in cubesA]
            vB = [c[:].rearrange("p (h w) -> p h w", w=WW) for c in cubesB]

            def mk_rhs0(off, t, half, di, dj):
                v = vA[t] if half == 0 else vB[t]
                return (wtiles[t * 9 + off],
                        v[:, di:di + 15, dj:dj + WO])

            banks(0, mk_rhs0)

            # ---- images 1..3: full tiles ----
            for im in range(1, BPC):
                src = xr[im - 1]
                cubes = feat(src, PIX, "i")
                views = [c[:].rearrange("p (h w) -> p h w", w=WW)
                         for c in cubes]

                def mk_rhs(off, t, half, di, dj, _v=views):
                    h0 = half * 15 + di
                    return (wtiles[t * 9 + off],
                            _v[t][:, h0:h0 + 15, dj:dj + WO])

                banks(im, mk_rhs)

    nc.compile()
    return nc


def _prep_weights(spline_kernel, scale_factor):
    """Truncated-power-folded weights, r-major (r, c) K layout,
    device chunk order t-major: chunk index = t*9 + off."""
    w = spline_kernel.astype(np.float64) * scale_factor.astype(np.float64)[:, None, :]
    cm = np.array([1.0, -4.0, 6.0, -4.0, 1.0], np.float64) / 6.0
    Wp = np.zeros((KH * KW, NFEAT, C, F), np.float64)
    wr = w.reshape(KH * KW, C, 8, F)
    for r in range(NR):
        for m in range(5):
            k = r - m
            if 0 <= k < 8:
                Wp[:, r] += wr[:, :, k] * cm[m]
    Wp[:, NR] = scale_factor.astype(np.float64).reshape(KH * KW, C, F)
    Wt = Wp.reshape(KH * KW, 3, 128, F)
    return np.ascontiguousarray(Wt.transpose(1, 0, 2, 3)).reshape(NMM, 128, F)


def _prep_static(spline_kernel, scale_factor, kan_bias, conv_bias):
    Wt = _prep_weights(spline_kernel, scale_factor)
    wt = np.ascontiguousarray(
        Wt.transpose(1, 0, 2).reshape(128, NMM * F), np.float32)

    consts = np.zeros((128, 8), np.float32)
    p = np.arange(128)
    for t in range(3):
        r = 4 * t + p // 32
        consts[:, t] = -(T0 + HGRID * r) / HGRID           # 5.5 - r
        consts[:, 3 + t] = NR - r                           # 11 - r
    consts[:, 6] = (kan_bias.astype(np.float64)
                    + conv_bias.astype(np.float64)).astype(np.float32)
    return wt, consts


def kernel(x, spline_kernel, scale_factor, kan_bias, conv_bias):
    from concourse import bass_utils

    x = np.asarray(x, np.float32)
    spline_kernel = np.asarray(spline_kernel, np.float32)
    scale_factor = np.asarray(scale_factor, np.float32)
    kan_bias = np.asarray(kan_bias, np.float32)
    conv_bias = np.asarray(conv_bias, np.float32)

    if "nc" not in _cache:
        _cache["nc"] = _build_program()
    nc = _cache["nc"]

    wt, consts = _prep_static(spline_kernel, scale_factor,
                              kan_bias, conv_bias)

    in_maps = []
    for c in range(N_CORES):
        xc = x[c * BPC:(c + 1) * BPC]                      # (4,32,32,32)
        xtc = np.ascontiguousarray(
            xc.transpose(3, 0, 1, 2).reshape(C, BPC * PIX), np.float32
        )
        xtr = np.ascontiguousarray(
            np.broadcast_to(xtc[None], (4, C, BPC * PIX))
            .reshape(128, BPC * PIX))
        in_maps.append({"xt": xtr, "wt": wt, "consts": consts})

    res = bass_utils.run_bass_kernel_spmd(
        nc, in_maps, core_ids=list(range(N_CORES)),
        **_cache.get("run_kwargs", {})
    )
    _cache["last_result"] = res

    out = np.empty((B, HO, WO, F), np.float32)
    for c in range(N_CORES):
        yc = res.results[c]["y"]                           # (128, 3600)
        out[c * BPC:(c + 1) * BPC] = (
            yc.reshape(F, BPC, HO, WO).transpose(1, 2, 3, 0)
        )
    return out


# revision 14
# speedup vs baseline: 1.0441x; 1.0441x over previous
"""Conv2D-KAN Trainium2 kernel (8-core data-parallel SPMD).

Formulation
-----------
The reference computes, per 3x3 patch (N = B*30*30 patches, in_size = 288):
    out[n,o] = sum_{i,k} sb[n,i,k] * (spline_kernel*scale)[i,k,o]
             + silu(xf) @ scale_factor + biases
where sb is a cubic B-spline basis (8 funcs) over a uniform grid
(knots t_r = -2.2 + 0.4 r, r = 0..11, h = 0.4).

Key identities:
 1. Basis values depend only on the underlying *pixel*, not the patch
    (patch extraction is a gather), so features are computed per pixel
    (8x less elementwise work than per-patch).
 2. Uniform cubic B-splines decompose over truncated powers:
        B_k(x) = (1/6) sum_{m=0..4} cm_m T_{k+m}(x), cm = [1,-4,6,-4,1]
        T_r(x) = min(relu((x - t_r)/h), 11-r)^3
    The clamp at 11-r makes every B_k *exactly* zero outside the grid
    (integer cancellation), matching the reference's out-of-range
    behaviour without masks, and T_11 == 0 so only r = 0..10 exist.
 3. The whole op is then a 3x3 convolution with 128 filters over
    pixel-feature channels (11 truncated cubes + silu per channel,
    blending folded into the weights), done as accumulating 128-K
    matmuls into PSUM banks of [128 filters, 450 patches].

Matmuls run in float32r (1 col/cycle at N>=256, vs 4 for fp32).
f32r's reduced mantissa interacts with the truncated-power
cancellation to give rel err ~1e-2 (< the 2e-2 gate; deterministic
for the fixed problem inputs).

Performance structure (per core: 4 images, 216 matmuls, 97.2K PE
cycles ~ 47us steady stream):
 * x is replicated 4x on the HOST -> one contiguous [128, 4KB] DMA
   per image (full-bandwidth 16-ring spray) instead of 12 small
   replica DMAs; the three per-image feature tiles read the same
   replicated tile with out-of-place relu.
 * All input DMAs are pre-issued at the top so no trigger ever sits
   behind a dependent output DMA in an engine queue.
 * Weights split 3F/15F/9F so the first chunks land before the first
   matmuls need them, without hogging ring bandwidth.
 * Image 0's tiles are built in two column chunks (rows 0..16 /
   15..31) so the first PSUM bank's matmuls start ~4us after the
   first 272KB of input lands.
"""

import sys

sys.path.insert(0, "/opt/trn_rl_repo")

import numpy as np

N_CORES = 8
B, HH, WW, C = 32, 32, 32, 32
F = 128
KH = KW = 3
HO, WO = HH - KH + 1, WW - KW + 1          # 30, 30
BPC = B // N_CORES                          # images per core = 4
PIX = HH * WW                               # 1024 pixels per image
NPC = BPC * HO * WO                         # 3600 patches per core
BANKN = 450                                 # patches per psum bank
HGRID = 0.4
T0 = -2.2                                   # first knot
NR = 11                                     # truncated-cube features
NFEAT = 12                                  # + silu
NMM = 27                                    # matmuls per bank
CA = 17 * WW                                # img-0 chunk A cols (rows 0..16)
CB = PIX - 15 * WW                          # chunk B cols (rows 15..31)

_cache = {}


def _build_program():
    import concourse.bacc as bacc
    import concourse.mybir as mybir
    import concourse.tile as tile

    f32 = mybir.dt.float32
    f32r = mybir.dt.float32r
    AF = mybir.ActivationFunctionType

    nc = bacc.Bacc("TRN2", target_bir_lowering=False, debug=False)
    # host-replicated input: 4 copies of the [32, BPC*PIX] image block
    xt = nc.dram_tensor("xt", [128, BPC * PIX], f32, kind="ExternalInput").ap()
    wt = nc.dram_tensor("wt", [128, NMM * F], f32r, kind="ExternalInput").ap()
    consts = nc.dram_tensor("consts", [128, 8], f32, kind="ExternalInput").ap()
    y = nc.dram_tensor("y", [F, NPC], f32, kind="ExternalOutput").ap()

    with tile.TileContext(nc) as tc:
        with (
            tc.tile_pool(name="wp", bufs=1) as wp,
            tc.tile_pool(name="cp", bufs=1) as cp,
            tc.tile_pool(name="xp", bufs=1) as xp,
            tc.tile_pool(name="fp", bufs=2) as fp,
            tc.tile_pool(name="sp", bufs=2) as sp,
            tc.tile_pool(name="op", bufs=1) as op_,
            tc.tile_pool(name="pp", bufs=4, space="PSUM") as pp,
        ):
            ct = cp.tile([128, 8], f32)
            nc.scalar.dma_start(ct[:], consts[:])

            # warm the ACT table set (silu's set also carries relu /
            # identity / square) before the first feature tile lands.
            warm = cp.tile([1, 1], f32, tag="warm")
            nc.scalar.activation(warm[:], ct[:1, :1], AF.Silu)

            # ---- all input DMAs pre-issued, priority order ----
            # sync HWDGE: image 0 in two chunks, then images 1..3
            xr = []
            x0a = xp.tile([128, CA], f32, tag="x0a")
            nc.sync.dma_start(x0a[:], xt[:, 0:CA])
            x0b = xp.tile([128, CB], f32, tag="x0b")
            nc.sync.dma_start(x0b[:], xt[:, 15 * WW:PIX])
            for im in range(1, BPC):
                xi = xp.tile([128, PIX], f32, tag=f"x{im}")
                nc.sync.dma_start(xi[:], xt[:, im * PIX:(im + 1) * PIX])
                xr.append(xi)
            # scalar HWDGE: t=0/t=1 weight chunks; gpsimd SWDGE: t=2
            w0 = wp.tile([128, 9 * F], f32r, tag="w0")
            nc.scalar.dma_start(w0[:], wt[:, :9 * F])
            wA = wp.tile([128, 9 * F], f32r, tag="wA")
            nc.scalar.dma_start(wA[:], wt[:, 9 * F:18 * F])
            wB = wp.tile([128, 9 * F], f32r, tag="wB")
            nc.gpsimd.dma_start(wB[:], wt[:, 18 * F:])
            wtiles = [w0[:, i * F:(i + 1) * F] for i in range(9)] + \
                     [wA[:, i * F:(i + 1) * F] for i in range(9)] + \
                     [wB[:, i * F:(i + 1) * F] for i in range(9)]

            # PE pre-warm: dummy bf16 matmuls on a memset tile keep the
            # tensor engine's clock gate open so the real stream starts
            # at full p-state. Zero-cost: PE is idle during the prologue.
            bf16 = mybir.dt.bfloat16
            wdum = cp.tile([128, 128], bf16, tag="wdum")
            nc.vector.memset(wdum[:], 0.0)
            psd = pp.tile([128, 128], f32, tag="psd")
            for _ in range(92):
                nc.tensor.matmul(psd[:], wdum[:], wdum[:],
                                 start=True, stop=True)

            out_t = op_.tile([F, NPC], f32)

            def feat(src, cols, ctag, sq_on_act):
                """relu/clamp/cube pipeline for one column range of the
                replicated tile. The square runs on ACT (which has
                slack) except on the latency-critical first chunk,
                where the pure-DVE chain is shorter."""
                cubes = []
                for t in range(3):
                    nsp = 128 if t < 2 else 96
                    ft = fp.tile([128, cols], f32, tag=f"f{t}{ctag}")
                    nc.scalar.activation(
                        ft[:nsp], src[:nsp], AF.Relu,
                        bias=ct[:nsp, t:t + 1], scale=1.0 / HGRID)
                    cube = sp.tile([128, cols], f32r, tag=f"c{t}{ctag}")
                    if t == 2:
                        nc.scalar.activation(
                            cube[96:128], src[96:128], AF.Silu)
                    nc.vector.tensor_scalar_min(
                        ft[:nsp], ft[:nsp], ct[:nsp, 3 + t:4 + t])
                    sq = fp.tile([128, cols], f32, tag=f"s{t}{ctag}")
                    if sq_on_act:
                        nc.scalar.activation(sq[:nsp], ft[:nsp], AF.Square)
                    else:
                        nc.vector.tensor_mul(sq[:nsp], ft[:nsp], ft[:nsp])
                    nc.vector.tensor_mul(cube[:nsp], sq[:nsp], ft[:nsp])
                    cubes.append(cube)
                return cubes

            def banks(im, mk_rhs):
                for half in range(2):
                    ps = pp.tile([F, BANKN], f32, tag="ps")
                    k = 0
                    # t-major: the first 9 matmuls only need feature
                    # tile 0, so PE starts before tiles 1/2 are built
                    for t in range(3):
                        for off in range(KH * KW):
                            di, dj = divmod(off, KW)
                            lhsT, rhs = mk_rhs(off, t, half, di, dj)
                            nc.tensor.matmul(
                                ps[:], lhsT, rhs,
                                start=(k == 0), stop=(k == NMM - 1),
                            )
                            k += 1
                    s = (im * 2 + half) * BANKN
                    if im == BPC - 1 and half == 1:
                        # last bank: act+DMA in halves so the final
                        # transfer starts as early as possible
                        hn = BANKN // 2
                        nc.scalar.activation(
                            out_t[:, s:s + hn], ps[:, :hn], AF.Identity,
                            bias=ct[:, 6:7], scale=1.0)
                        nc.sync.dma_start(
                            y[:, s:s + hn], out_t[:, s:s + hn])
                        nc.scalar.activation(
                            out_t[:, s + hn:s + BANKN], ps[:, hn:],
                            AF.Identity, bias=ct[:, 6:7], scale=1.0)
                        nc.scalar.dma_start(
                            y[:, s + hn:s + BANKN],
                            out_t[:, s + hn:s + BANKN])
                    else:
                        nc.scalar.activation(
                            out_t[:, s:s + BANKN], ps[:], AF.Identity,
                            bias=ct[:, 6:7], scale=1.0,
                        )
                        nc.sync.dma_start(
                            y[:, s:s + BANKN], out_t[:, s:s + BANKN])

            # ---- image 0: chunked (A = rows 0..16, B = rows 15..31) ----
            cubesA = feat(x0a, CA, "a", False)
            cubesB = feat(x0b, CB, "b", True)
            vA = [c[:].rearrange("p (h w) -> p h w", w=WW) for c in cubesA]
            vB = [c[:].rearrange("p (h w) -> p h w", w=WW) for c in cubesB]

            def mk_rhs0(off, t, half, di, dj):
                v = vA[t] if half == 0 else vB[t]
                return (wtiles[t * 9 + off],
                        v[:, di:di + 15, dj:dj + WO])

            banks(0, mk_rhs0)

            # ---- images 1..3: full tiles ----
            for im in range(1, BPC):
                src = xr[im - 1]
                cubes = feat(src, PIX, "i", True)
                views = [c[:].rearrange("p (h w) -> p h w", w=WW)
                         for c in cubes]

                def mk_rhs(off, t, half, di, dj, _v=views):
                    h0 = half * 15 + di
                    return (wtiles[t * 9 + off],
                            _v[t][:, h0:h0 + 15, dj:dj + WO])

                banks(im, mk_rhs)

    nc.compile()
    return nc


def _prep_weights(spline_kernel, scale_factor):
    """Truncated-power-folded weights, r-major (r, c) K layout,
    device chunk order t-major: chunk index = t*9 + off."""
    w = spline_kernel.astype(np.float64) * scale_factor.astype(np.float64)[:, None, :]
    cm = np.array([1.0, -4.0, 6.0, -4.0, 1.0], np.float64) / 6.0
    Wp = np.zeros((KH * KW, NFEAT, C, F), np.float64)
    wr = w.reshape(KH * KW, C, 8, F)
    for r in range(NR):
        for m in range(5):
            k = r - m
            if 0 <= k < 8:
                Wp[:, r] += wr[:, :, k] * cm[m]
    Wp[:, NR] = scale_factor.astype(np.float64).reshape(KH * KW, C, F)
    Wt = Wp.reshape(KH * KW, 3, 128, F)
    return np.ascontiguousarray(Wt.transpose(1, 0, 2, 3)).reshape(NMM, 128, F)


def _prep_static(spline_kernel, scale_factor, kan_bias, conv_bias):
    Wt = _prep_weights(spline_kernel, scale_factor)
    wt = np.ascontiguousarray(
        Wt.transpose(1, 0, 2).reshape(128, NMM * F), np.float32)

    consts = np.zeros((128, 8), np.float32)
    p = np.arange(128)
    for t in range(3):
        r = 4 * t + p // 32
        consts[:, t] = -(T0 + HGRID * r) / HGRID           # 5.5 - r
        consts[:, 3 + t] = NR - r                           # 11 - r
    consts[:, 6] = (kan_bias.astype(np.float64)
                    + conv_bias.astype(np.float64)).astype(np.float32)
    return wt, consts


def kernel(x, spline_kernel, scale_factor, kan_bias, conv_bias):
    from concourse import bass_utils

    x = np.asarray(x, np.float32)
    spline_kernel = np.asarray(spline_kernel, np.float32)
    scale_factor = np.asarray(scale_factor, np.float32)
    kan_bias = np.asarray(kan_bias, np.float32)
    conv_bias = np.asarray(conv_bias, np.float32)

    if "nc" not in _cache:
        _cache["nc"] = _build_program()
    nc = _cache["nc"]

    wt, consts = _prep_static(spline_kernel, scale_factor,
                              kan_bias, conv_bias)

    in_maps = []
    for c in range(N_CORES):
        xc = x[c * BPC:(c + 1) * BPC]                      # (4,32,32,32)
        xtc = np.ascontiguousarray(
            xc.transpose(3, 0, 1, 2).reshape(C, BPC * PIX), np.float32
        )
        xtr = np.ascontiguousarray(
            np.broadcast_to(xtc[None], (4, C, BPC * PIX))
            .reshape(128, BPC * PIX))
        in_maps.append({"xt": xtr, "wt": wt, "consts": consts})

    res = bass_utils.run_bass_kernel_spmd(
        nc, in_maps, core_ids=list(range(N_CORES)),
        **_cache.get("run_kwargs", {})
    )
    _cache["last_result"] = res

    out = np.empty((B, HO, WO, F), np.float32)
    for c in range(N_CORES):
        yc = res.results[c]["y"]                           # (128, 3600)
        out[c * BPC:(c + 1) * BPC] = (
            yc.reshape(F, BPC, HO, WO).transpose(1, 2, 3, 0)
        )
    return out
